# revision 39
# baseline (speedup 1.0000x reference)
"""BinaryConv2D Trainium2 kernel — FP8 DoubleRow version.

Reference computation:
    out = conv2d(sign(x), sign(w), SAME, stride 1)   # sign(v) = +1 if v>=0 else -1
    x: (64, 56, 56, 128) f32, w: (3, 3, 128, 256) f32 -> out (64, 56, 56, 256) f32

Strategy (data-parallel over batch, 8 images per NeuronCore):
  1. Per image: SWDGE cast-DMA x f32 -> bf16 (HBM->HBM), then HW xbar
     DMA-transpose -> SBUF [128 ci, 3136 px] bf16.
  2. DVE binarize into TWO zero-padded fp8 planes per image (values +-0.5,
     weights are scaled +-2 so products are exactly +-1):
       A[r, c] = sign(x[r-1, c-1]) / 2    (the standard SAME-pad layout)
       B[r, c] = sign(x[r-1, c])   / 2    (shifted copy, for horizontal pairs)
     Plane rows are PW=64 wide so vertical tap offsets are 16-byte aligned
     (a DoubleRow AP requirement).
  3. Conv: weights STATIONARY in fp8 DoubleRow mode — each matmul contracts
     2 taps x 128 ci at once.  9 taps -> 4 DoubleRow pairs + 1 normal fp8
     matmul, accumulated in PSUM f32 [128 co_half, 512 px] per 8-row group:
       P0..P2: (0,dj)+(1,dj)  vertical pairs, moving-AP pair step 64 B
       P3:     (2,0)@A+(2,1)@B, pair step 3840 B (A->B plane offset)
       P4:     (2,2) normal fp8 matmul
     Moving operand = overlapping strided AP [128, 2, 512] over the plane.
  4. Output comes out transposed ([co, px]); Scalar engine copies PSUM ->
     bf16 (exact: |out| <= 512 in practice), then PE identity-matmul
     transposes flip each [co 128, px 112] chunk to [px, co]; DVE/ACT copy
     to an f32 stage and 4 DMAs per (image, co_half) write NHWC output.

Built on bacc.Bacc so multi-semaphore waits are legalized into
EventSemaphore chains.
"""

import sys

if "/opt/trn_rl_repo" not in sys.path:
    sys.path.insert(0, "/opt/trn_rl_repo")

import numpy as np
import bass_rust

import concourse.bacc as bacc
import concourse.bass as bass
import concourse.mybir as mybir
from concourse.tile import TileContext
from concourse.bass_utils import run_bass_kernel_spmd

N_CORES = 8
IMGS = 8  # images per core
H = W = 56
C = 128  # input channels (= contraction dim = SBUF partitions)
O = 256  # output channels
PW = 64  # padded row width (16-aligned for DoubleRow pair steps)
PROWS = 60  # 58 padded rows + 2 zero guard rows for tap reads past the end
PLANE = PROWS * PW  # 3840 bytes per partition per plane
GROUPS = 7  # 8-output-row groups per image
GW = 8 * PW  # 512 moving pixels per group
F32 = mybir.dt.float32
BF16 = mybir.dt.bfloat16
FP8 = mybir.dt.float8e4
DR = mybir.MatmulPerfMode.DoubleRow

# DoubleRow tap pairs (di, dj) and the single leftover tap
PAIRS = [((0, 0), (1, 0)), ((0, 1), (1, 1)), ((0, 2), (1, 2)), ((2, 0), (2, 1))]
SINGLE = (2, 2)


def _pair_rhs(plane_ap, off: int, step: int, n: int):
    """Overlapping 3D moving AP [128, 2, n]: two tap windows `step` bytes
    apart, each n contiguous fp8 pixels starting at `off` within the tile."""
    return bass_rust.AP(
        tensor=plane_ap.tensor,
        offset=plane_ap.offset + off,
        ap=[[plane_ap.ap[0][0], plane_ap.ap[0][1]], [step, 2], [1, n]],
    )


def build_nc() -> bass.Bass:
    nc = bacc.Bacc()
    x_t = nc.dram_tensor("x", [IMGS, H * W, C], F32, kind="ExternalInput")
    wq_t = nc.dram_tensor("wq", [C, 2304], FP8, kind="ExternalInput")
    id_t = nc.dram_tensor("ident", [128, 128], BF16, kind="ExternalInput")
    y_t = nc.dram_tensor("out", [IMGS, H, W, O], F32, kind="ExternalOutput")
    # per-image bf16 bounce tensors for the cast + xbar-transpose input path
    xb_ts = [nc.dram_tensor(f"xb{i}", [H * W, C], BF16) for i in range(IMGS)]

    with TileContext(nc) as tc:
        with (
            tc.tile_pool(name="const", bufs=1) as constp,
            tc.tile_pool(name="xtr", bufs=4) as xtrp,
            tc.tile_pool(name="out1", bufs=4) as out1p,
            tc.tile_pool(name="stage", bufs=3) as stagep,
            tc.tile_pool(name="psum1", bufs=6, space="PSUM") as psum1p,
            tc.tile_pool(name="psum2", bufs=2, space="PSUM") as psum2p,
        ):
            wq = constp.tile([C, 2304], FP8)
            nc.sync.dma_start(out=wq[:], in_=wq_t[:])
            identb = constp.tile([128, 128], BF16)
            nc.sync.dma_start(out=identb[:], in_=id_t[:])

            planes = []
            for i in range(IMGS):
                plane = constp.tile([C, 2 * PLANE], FP8, tag=f"plane{i}")
                planes.append(plane)

            xtrs = {}
            HALF = H * W // 2  # 1568 px = 28 image rows

            def prep_input(i, halves=False):
                """cast-DMA + xbar transpose image i (non-blocking queues).
                halves=True splits into two half-image pipelines so the first
                matmuls can start sooner (image 0 startup latency)."""
                if halves:
                    nc.gpsimd.dma_start(out=xb_ts[i][0:HALF], in_=x_t[i][0:HALF])
                    nc.gpsimd.dma_start(
                        out=xb_ts[i][HALF : H * W], in_=x_t[i][HALF : H * W]
                    )
                    xtra = xtrp.tile([C, HALF], BF16)
                    nc.sync.dma_start(
                        out=xtra[:], in_=xb_ts[i][0:HALF], transpose=True
                    )
                    xtrb = xtrp.tile([C, HALF], BF16)
                    nc.sync.dma_start(
                        out=xtrb[:], in_=xb_ts[i][HALF : H * W], transpose=True
                    )
                    xtrs[i] = (xtra, xtrb)
                    return
                nc.gpsimd.dma_start(out=xb_ts[i][:], in_=x_t[i])
                xtr = xtrp.tile([C, H * W], BF16)
                nc.sync.dma_start(out=xtr[:], in_=xb_ts[i][:], transpose=True)
                xtrs[i] = xtr

            def prep_plane(i):
                """Pad-zero + binarize image i, split across DVE (plane A)
                and ACT (plane B) so the two run concurrently.  Only pad
                regions that valid outputs actually read are zeroed; junk
                columns 58+ and the guard rows only ever feed discarded
                output columns, so stale SBUF there is harmless."""
                xtr = xtrs.pop(i)
                pl = planes[i]
                intA = pl[:, PW : PW + 56 * PW].rearrange("c (r w) -> c r w", w=PW)
                intB = pl[:, PLANE + PW : PLANE + PW + 56 * PW].rearrange(
                    "c (r w) -> c r w", w=PW
                )
                # A pads on DVE: top row, bottom rows + guards, side cols
                nc.vector.memset(pl[:, 0:PW], 0.0)
                nc.vector.memset(pl[:, 57 * PW : PLANE], 0.0)
                nc.vector.memset(intA[:, :, 0:1], 0.0)
                nc.vector.memset(intA[:, :, 57:64], 0.0)
                # B pads on ACT (MemsetZero still reads its input AP, so
                # feed it a broadcast of the always-initialized identity)
                def act_zero(dst):
                    ib = identb[:]
                    bc = bass_rust.AP(
                        tensor=ib.tensor,
                        offset=ib.offset,
                        ap=[[ib.ap[0][0], dst.ap[0][1]]]
                        + [[0, n] for _, n in dst.ap[1:]],
                    )
                    nc.scalar.activation(
                        dst, bc, mybir.ActivationFunctionType.Copy, scale=0.0
                    )

                act_zero(pl[:, PLANE + 57 * PW : 2 * PLANE])
                act_zero(intB[:, :, 56:64])
                act_zero(pl[:, PLANE : PLANE + PW])

                pieces = (
                    [(xtr, 0, H)]
                    if not isinstance(xtr, tuple)
                    else [(xtr[0], 0, H // 2), (xtr[1], H // 2, H)]
                )
                deferred = []
                for n, (piece, r0, r1) in enumerate(pieces):
                    src = piece[:].rearrange("c (r w) -> c r w", w=W)
                    nc.vector.tensor_scalar(
                        intA[:, r0:r1, 1 : 1 + W],
                        src,
                        0.0,
                        0.5,
                        op0=mybir.AluOpType.is_ge,
                        op1=mybir.AluOpType.subtract,
                    )

                    # plane B binarize on the Scalar engine: Sign gives +-1
                    # (not +-0.5), so the B-side tap weights are +-1
                    def sign_b(r0=r0, r1=r1, src=src):
                        nc.scalar.activation(
                            intB[:, r0:r1, 0:W],
                            src,
                            mybir.ActivationFunctionType.Sign,
                        )

                    if n == 0:
                        sign_b()
                    else:
                        # second half emitted after the first conv group's
                        # PSUM drain, so its xbar wait can't head-of-line
                        # block the ACT queue (it isn't needed until g3)
                        deferred.append(sign_b)
                return deferred

            prep_input(0, halves=True)
            prep_input(1)
            deferred_b = prep_plane(0)

            for i in range(IMGS):
                if i + 2 < IMGS:
                    prep_input(i + 2)
                pl_ap = planes[i][:]
                for h in range(2):
                    if h == 0 and i + 1 < IMGS:
                        prep_plane(i + 1)
                    stage = stagep.tile([128, 28 * 128], F32)
                    st3 = stage[:].rearrange("p (t o) -> p t o", o=128)
                    pending = None  # (g, out1 tile) awaiting transpose

                    def flush_transposes():
                        nonlocal pending
                        if pending is None:
                            return
                        g, o1 = pending
                        pending = None
                        ps2 = psum2p.tile([128, GW], BF16)
                        for c in range(4):
                            nc.tensor.transpose(
                                ps2[:, c * 128 : (c + 1) * 128],
                                o1[:, c * 128 : (c + 1) * 128],
                                identb[:],
                            )
                        # one whole-group drain (DVE); psum1 drains are ACT
                        nc.vector.tensor_copy(
                            stage[:, g * GW : (g + 1) * GW], ps2[:]
                        )

                    last = i == IMGS - 1 and h == 1

                    def out_dmas(c0, c1):
                        # (even/odd rows) x chunk range; on gpsimd to keep
                        # the sync queue free for the input xbar transposes.
                        # On the very last half-image the xbars are done, so
                        # split across both queues to drain the tail faster.
                        ye = y_t[i].rearrange("(r2 two) w c -> two w r2 c", two=2)
                        cs = slice(c0, c1)
                        eng0 = nc.sync if last else nc.gpsimd
                        eng0.dma_start(
                            out=ye[0][:, cs, h * 128 : (h + 1) * 128],
                            in_=st3[0:W, cs, :],
                        )
                        nc.gpsimd.dma_start(
                            out=ye[1][:, cs, h * 128 : (h + 1) * 128],
                            in_=st3[64 : 64 + W, cs, :],
                        )
                    for g in range(GROUPS):
                        ps1 = psum1p.tile([128, GW], F32)
                        base = PW * 8 * g
                        for p in range(len(PAIRS)):
                            lhsT = wq[
                                :, (p * 2 + h) * 256 : (p * 2 + h) * 256 + 256
                            ].rearrange("c (t m) -> c t m", t=2)
                            if p < 3:
                                rhs = _pair_rhs(pl_ap, base + p, PW, GW)
                            else:
                                rhs = _pair_rhs(pl_ap, base + 2 * PW, PLANE, GW)
                            nc.tensor.matmul(
                                ps1[:], lhsT, rhs, start=(p == 0), stop=False,
                                perf_mode=DR,
                            )
                        nc.tensor.matmul(
                            ps1[:],
                            wq[:, 2048 + h * 128 : 2048 + (h + 1) * 128],
                            pl_ap[:, base + 2 * PW + 2 : base + 2 * PW + 2 + GW],
                            start=False,
                            stop=True,
                        )
                        o1 = out1p.tile([128, GW], BF16)
                        nc.scalar.activation(
                            o1[:], ps1[:], mybir.ActivationFunctionType.Copy
                        )
                        if deferred_b and i == 0 and h == 0:
                            deferred_b.pop()()
                        flush_transposes()
                        pending = (g, o1)
                        # drain staged chunks early; finer-grained on the
                        # very last half-image to shorten the kernel tail
                        if last and g in (2, 4, 6):
                            out_dmas(7 * (g // 2 - 1), 7 * (g // 2))
                        elif not last and g == 5:
                            out_dmas(0, 14)
                    flush_transposes()
                    if last:
                        out_dmas(21, 28)
                    else:
                        out_dmas(14, 28)

    nc.finalize()
    return nc


_NC_CACHE = None


def _get_nc():
    global _NC_CACHE
    if _NC_CACHE is None:
        _NC_CACHE = build_nc()
    return _NC_CACHE


def prep_wq(w: np.ndarray) -> np.ndarray:
    """Binarize weights to +-2 fp8 (inputs are +-0.5 -> products +-1),
    laid out [ci, pair/co_half/tap/co_low] for DoubleRow stationary loads."""
    import ml_dtypes

    wb = np.where(w >= 0, np.float32(2.0), np.float32(-2.0))  # [3,3,128,256]
    wb1 = np.where(w >= 0, np.float32(1.0), np.float32(-1.0))
    cols = np.zeros((C, 2304), np.float32)
    for p, (t0, t1) in enumerate(PAIRS):
        # pair 3's second tap reads plane B, whose values are +-1 (Sign
        # activation), not +-0.5 -> its weights stay +-1
        w1 = wb1 if p == 3 else wb
        for h in range(2):
            base = (p * 2 + h) * 256
            cols[:, base : base + 128] = wb[t0[0], t0[1], :, h * 128 : (h + 1) * 128]
            cols[:, base + 128 : base + 256] = w1[
                t1[0], t1[1], :, h * 128 : (h + 1) * 128
            ]
    for h in range(2):
        cols[:, 2048 + h * 128 : 2048 + (h + 1) * 128] = wb[
            SINGLE[0], SINGLE[1], :, h * 128 : (h + 1) * 128
        ]
    return np.ascontiguousarray(cols.astype(ml_dtypes.float8_e4m3))


def prep_ident() -> np.ndarray:
    import ml_dtypes

    return np.eye(128, dtype=np.float32).astype(ml_dtypes.bfloat16)


def _ntff_hook():
    """NTFF capture context manager via the axon PJRT .so."""
    sys.path.insert(0, "/root/.axon_site")
    from trn_agent_boot.trn_boot import _ntff_profile_via_ctypes

    return _ntff_profile_via_ctypes("/opt/axon/libaxon_pjrt.so")


def run(inputs: dict, profile_dir: str | None = None):
    """Run on all 8 NeuronCores. Returns (full_output, BassKernelResults)."""
    x = np.ascontiguousarray(np.asarray(inputs["x"], dtype=np.float32))
    w = np.ascontiguousarray(np.asarray(inputs["w"], dtype=np.float32))
    assert x.shape == (N_CORES * IMGS, H, W, C), x.shape
    assert w.shape == (3, 3, C, O), w.shape

    nc = _get_nc()
    wq = prep_wq(w)
    ident = prep_ident()
    xr = x.reshape(N_CORES, IMGS, H * W, C)
    in_maps = [{"x": xr[i], "wq": wq, "ident": ident} for i in range(N_CORES)]
    if profile_dir is not None:
        hook = _ntff_hook()
        with hook(profile_dir, [0]):
            res = run_bass_kernel_spmd(nc, in_maps, list(range(N_CORES)))
    else:
        res = run_bass_kernel_spmd(nc, in_maps, list(range(N_CORES)))
    out = np.concatenate([res.results[i]["out"] for i in range(N_CORES)], axis=0)
    return out, res


def kernel(**inputs: np.ndarray) -> np.ndarray:
    out, _ = run(inputs)
    return out


# revision 40
# speedup vs baseline: 1.0301x; 1.0301x over previous
"""BinaryConv2D Trainium2 kernel — FP8 DoubleRow version.

Reference computation:
    out = conv2d(sign(x), sign(w), SAME, stride 1)   # sign(v) = +1 if v>=0 else -1
    x: (64, 56, 56, 128) f32, w: (3, 3, 128, 256) f32 -> out (64, 56, 56, 256) f32

Strategy (data-parallel over batch, 8 images per NeuronCore):
  1. Per image: SWDGE cast-DMA x f32 -> bf16 (HBM->HBM), then HW xbar
     DMA-transpose -> SBUF [128 ci, 3136 px] bf16.
  2. DVE binarize into TWO zero-padded fp8 planes per image (values +-0.5,
     weights are scaled +-2 so products are exactly +-1):
       A[r, c] = sign(x[r-1, c-1]) / 2    (the standard SAME-pad layout)
       B[r, c] = sign(x[r-1, c])   / 2    (shifted copy, for horizontal pairs)
     Plane rows are PW=64 wide so vertical tap offsets are 16-byte aligned
     (a DoubleRow AP requirement).
  3. Conv: weights STATIONARY in fp8 DoubleRow mode — each matmul contracts
     2 taps x 128 ci at once.  9 taps -> 4 DoubleRow pairs + 1 normal fp8
     matmul, accumulated in PSUM f32 [128 co_half, 512 px] per 8-row group:
       P0..P2: (0,dj)+(1,dj)  vertical pairs, moving-AP pair step 64 B
       P3:     (2,0)@A+(2,1)@B, pair step 3840 B (A->B plane offset)
       P4:     (2,2) normal fp8 matmul
     Moving operand = overlapping strided AP [128, 2, 512] over the plane.
  4. Output comes out transposed ([co, px]); Scalar engine copies PSUM ->
     bf16 (exact: |out| <= 512 in practice), then PE identity-matmul
     transposes flip each [co 128, px 112] chunk to [px, co]; DVE/ACT copy
     to an f32 stage and 4 DMAs per (image, co_half) write NHWC output.

Built on bacc.Bacc so multi-semaphore waits are legalized into
EventSemaphore chains.
"""

import sys

if "/opt/trn_rl_repo" not in sys.path:
    sys.path.insert(0, "/opt/trn_rl_repo")

import numpy as np
import bass_rust

import concourse.bacc as bacc
import concourse.bass as bass
import concourse.mybir as mybir
from concourse.tile import TileContext
from concourse.bass_utils import run_bass_kernel_spmd

N_CORES = 8
IMGS = 8  # images per core
H = W = 56
C = 128  # input channels (= contraction dim = SBUF partitions)
O = 256  # output channels
PW = 64  # padded row width (16-aligned for DoubleRow pair steps)
PROWS = 60  # 58 padded rows + 2 zero guard rows for tap reads past the end
PLANE = PROWS * PW  # 3840 bytes per partition per plane
GROUPS = 7  # 8-output-row groups per image
GW = 8 * PW  # 512 moving pixels per group
F32 = mybir.dt.float32
BF16 = mybir.dt.bfloat16
FP8 = mybir.dt.float8e4
DR = mybir.MatmulPerfMode.DoubleRow

# DoubleRow tap pairs (di, dj) and the single leftover tap
PAIRS = [((0, 0), (1, 0)), ((0, 1), (1, 1)), ((0, 2), (1, 2)), ((2, 0), (2, 1))]
SINGLE = (2, 2)


def _pair_rhs(plane_ap, off: int, step: int, n: int):
    """Overlapping 3D moving AP [128, 2, n]: two tap windows `step` bytes
    apart, each n contiguous fp8 pixels starting at `off` within the tile."""
    return bass_rust.AP(
        tensor=plane_ap.tensor,
        offset=plane_ap.offset + off,
        ap=[[plane_ap.ap[0][0], plane_ap.ap[0][1]], [step, 2], [1, n]],
    )


def build_nc() -> bass.Bass:
    nc = bacc.Bacc()
    x_t = nc.dram_tensor("x", [IMGS, H * W, C], F32, kind="ExternalInput")
    wq_t = nc.dram_tensor("wq", [C, 2304], FP8, kind="ExternalInput")
    id_t = nc.dram_tensor("ident", [128, 128], BF16, kind="ExternalInput")
    y_t = nc.dram_tensor("out", [IMGS, H, W, O], F32, kind="ExternalOutput")
    # per-image bf16 bounce tensors for the cast + xbar-transpose input path
    xb_ts = [nc.dram_tensor(f"xb{i}", [H * W, C], BF16) for i in range(IMGS)]

    with TileContext(nc) as tc:
        with (
            tc.tile_pool(name="const", bufs=1) as constp,
            tc.tile_pool(name="xtr", bufs=4) as xtrp,
            tc.tile_pool(name="out1", bufs=4) as out1p,
            tc.tile_pool(name="stage", bufs=3) as stagep,
            tc.tile_pool(name="psum1", bufs=6, space="PSUM") as psum1p,
            tc.tile_pool(name="psum2", bufs=2, space="PSUM") as psum2p,
        ):
            wq = constp.tile([C, 2304], FP8)
            nc.sync.dma_start(out=wq[:], in_=wq_t[:])
            identb = constp.tile([128, 128], BF16)
            nc.sync.dma_start(out=identb[:], in_=id_t[:])

            planes = []
            for i in range(IMGS):
                plane = constp.tile([C, 2 * PLANE], FP8, tag=f"plane{i}")
                planes.append(plane)

            xtrs = {}
            HALF = H * W // 2  # 1568 px = 28 image rows

            def prep_input(i, halves=False):
                """cast-DMA + xbar transpose image i (non-blocking queues).
                halves=True splits into two half-image pipelines so the first
                matmuls can start sooner (image 0 startup latency)."""
                if halves:
                    nc.gpsimd.dma_start(out=xb_ts[i][0:HALF], in_=x_t[i][0:HALF])
                    nc.gpsimd.dma_start(
                        out=xb_ts[i][HALF : H * W], in_=x_t[i][HALF : H * W]
                    )
                    xtra = xtrp.tile([C, HALF], BF16)
                    nc.sync.dma_start(
                        out=xtra[:], in_=xb_ts[i][0:HALF], transpose=True
                    )
                    xtrb = xtrp.tile([C, HALF], BF16)
                    nc.sync.dma_start(
                        out=xtrb[:], in_=xb_ts[i][HALF : H * W], transpose=True
                    )
                    xtrs[i] = (xtra, xtrb)
                    return
                nc.gpsimd.dma_start(out=xb_ts[i][:], in_=x_t[i])
                xtr = xtrp.tile([C, H * W], BF16)
                nc.sync.dma_start(out=xtr[:], in_=xb_ts[i][:], transpose=True)
                xtrs[i] = xtr

            def prep_plane(i):
                """Pad-zero + binarize image i, split across DVE (plane A)
                and ACT (plane B) so the two run concurrently.  Only pad
                regions that valid outputs actually read are zeroed; junk
                columns 58+ and the guard rows only ever feed discarded
                output columns, so stale SBUF there is harmless."""
                xtr = xtrs.pop(i)
                pl = planes[i]
                intA = pl[:, PW : PW + 56 * PW].rearrange("c (r w) -> c r w", w=PW)
                intB = pl[:, PLANE + PW : PLANE + PW + 56 * PW].rearrange(
                    "c (r w) -> c r w", w=PW
                )
                # A pads on DVE: top row, bottom rows + guards, side cols
                nc.vector.memset(pl[:, 0:PW], 0.0)
                nc.vector.memset(pl[:, 57 * PW : PLANE], 0.0)
                nc.vector.memset(intA[:, :, 0:1], 0.0)
                nc.vector.memset(intA[:, :, 57:64], 0.0)
                # B pads on ACT (MemsetZero still reads its input AP, so
                # feed it a broadcast of the always-initialized identity)
                def act_zero(dst):
                    ib = identb[:]
                    bc = bass_rust.AP(
                        tensor=ib.tensor,
                        offset=ib.offset,
                        ap=[[ib.ap[0][0], dst.ap[0][1]]]
                        + [[0, n] for _, n in dst.ap[1:]],
                    )
                    nc.scalar.activation(
                        dst, bc, mybir.ActivationFunctionType.Copy, scale=0.0
                    )

                act_zero(pl[:, PLANE + 57 * PW : 2 * PLANE])
                act_zero(intB[:, :, 56:64])
                act_zero(pl[:, PLANE : PLANE + PW])

                pieces = (
                    [(xtr, 0, H)]
                    if not isinstance(xtr, tuple)
                    else [(xtr[0], 0, H // 2), (xtr[1], H // 2, H)]
                )
                for piece, r0, r1 in pieces:
                    src = piece[:].rearrange("c (r w) -> c r w", w=W)
                    nc.vector.tensor_scalar(
                        intA[:, r0:r1, 1 : 1 + W],
                        src,
                        0.0,
                        0.5,
                        op0=mybir.AluOpType.is_ge,
                        op1=mybir.AluOpType.subtract,
                    )
                    # plane B binarize on the Scalar engine: Sign gives +-1
                    # (not +-0.5), so the B-side tap weights are +-1
                    nc.scalar.activation(
                        intB[:, r0:r1, 0:W],
                        src,
                        mybir.ActivationFunctionType.Sign,
                    )

            prep_input(0, halves=True)
            prep_input(1)
            prep_plane(0)

            for i in range(IMGS):
                if i + 2 < IMGS:
                    prep_input(i + 2)
                pl_ap = planes[i][:]
                for h in range(2):
                    if h == 0 and i + 2 < IMGS:
                        prep_plane(i + 2)
                    if h == 1 and i == 0:
                        prep_plane(1)
                    stage = stagep.tile([128, 28 * 128], F32)
                    st3 = stage[:].rearrange("p (t o) -> p t o", o=128)
                    pending = None  # (g, out1 tile) awaiting transpose

                    def flush_transposes():
                        nonlocal pending
                        if pending is None:
                            return
                        g, o1 = pending
                        pending = None
                        ps2 = psum2p.tile([128, GW], BF16)
                        for c in range(4):
                            nc.tensor.transpose(
                                ps2[:, c * 128 : (c + 1) * 128],
                                o1[:, c * 128 : (c + 1) * 128],
                                identb[:],
                            )
                        # one whole-group drain (DVE); psum1 drains are ACT
                        nc.vector.tensor_copy(
                            stage[:, g * GW : (g + 1) * GW], ps2[:]
                        )

                    def out_dmas(c0, c1):
                        # (even/odd rows) x chunk range; on gpsimd to keep
                        # the sync queue free for the input xbar transposes
                        ye = y_t[i].rearrange("(r2 two) w c -> two w r2 c", two=2)
                        cs = slice(c0, c1)
                        nc.gpsimd.dma_start(
                            out=ye[0][:, cs, h * 128 : (h + 1) * 128],
                            in_=st3[0:W, cs, :],
                        )
                        nc.gpsimd.dma_start(
                            out=ye[1][:, cs, h * 128 : (h + 1) * 128],
                            in_=st3[64 : 64 + W, cs, :],
                        )

                    last = i == IMGS - 1 and h == 1
                    for g in range(GROUPS):
                        ps1 = psum1p.tile([128, GW], F32)
                        base = PW * 8 * g
                        for p in range(len(PAIRS)):
                            lhsT = wq[
                                :, (p * 2 + h) * 256 : (p * 2 + h) * 256 + 256
                            ].rearrange("c (t m) -> c t m", t=2)
                            if p < 3:
                                rhs = _pair_rhs(pl_ap, base + p, PW, GW)
                            else:
                                rhs = _pair_rhs(pl_ap, base + 2 * PW, PLANE, GW)
                            nc.tensor.matmul(
                                ps1[:], lhsT, rhs, start=(p == 0), stop=False,
                                perf_mode=DR,
                            )
                        nc.tensor.matmul(
                            ps1[:],
                            wq[:, 2048 + h * 128 : 2048 + (h + 1) * 128],
                            pl_ap[:, base + 2 * PW + 2 : base + 2 * PW + 2 + GW],
                            start=False,
                            stop=True,
                        )
                        o1 = out1p.tile([128, GW], BF16)
                        nc.scalar.activation(
                            o1[:], ps1[:], mybir.ActivationFunctionType.Copy
                        )
                        flush_transposes()
                        pending = (g, o1)
                        # drain staged chunks early; finer-grained on the
                        # very last half-image to shorten the kernel tail
                        if last and g in (2, 4, 6):
                            out_dmas(7 * (g // 2 - 1), 7 * (g // 2))
                        elif not last and g == 5:
                            out_dmas(0, 14)
                    flush_transposes()
                    if last:
                        out_dmas(21, 28)
                    else:
                        out_dmas(14, 28)

    nc.finalize()
    return nc


_NC_CACHE = None


def _get_nc():
    global _NC_CACHE
    if _NC_CACHE is None:
        _NC_CACHE = build_nc()
    return _NC_CACHE


def prep_wq(w: np.ndarray) -> np.ndarray:
    """Binarize weights to +-2 fp8 (inputs are +-0.5 -> products +-1),
    laid out [ci, pair/co_half/tap/co_low] for DoubleRow stationary loads."""
    import ml_dtypes

    wb = np.where(w >= 0, np.float32(2.0), np.float32(-2.0))  # [3,3,128,256]
    wb1 = np.where(w >= 0, np.float32(1.0), np.float32(-1.0))
    cols = np.zeros((C, 2304), np.float32)
    for p, (t0, t1) in enumerate(PAIRS):
        # pair 3's second tap reads plane B, whose values are +-1 (Sign
        # activation), not +-0.5 -> its weights stay +-1
        w1 = wb1 if p == 3 else wb
        for h in range(2):
            base = (p * 2 + h) * 256
            cols[:, base : base + 128] = wb[t0[0], t0[1], :, h * 128 : (h + 1) * 128]
            cols[:, base + 128 : base + 256] = w1[
                t1[0], t1[1], :, h * 128 : (h + 1) * 128
            ]
    for h in range(2):
        cols[:, 2048 + h * 128 : 2048 + (h + 1) * 128] = wb[
            SINGLE[0], SINGLE[1], :, h * 128 : (h + 1) * 128
        ]
    return np.ascontiguousarray(cols.astype(ml_dtypes.float8_e4m3))


def prep_ident() -> np.ndarray:
    import ml_dtypes

    return np.eye(128, dtype=np.float32).astype(ml_dtypes.bfloat16)


def _ntff_hook():
    """NTFF capture context manager via the axon PJRT .so."""
    sys.path.insert(0, "/root/.axon_site")
    from trn_agent_boot.trn_boot import _ntff_profile_via_ctypes

    return _ntff_profile_via_ctypes("/opt/axon/libaxon_pjrt.so")


def run(inputs: dict, profile_dir: str | None = None):
    """Run on all 8 NeuronCores. Returns (full_output, BassKernelResults)."""
    x = np.ascontiguousarray(np.asarray(inputs["x"], dtype=np.float32))
    w = np.ascontiguousarray(np.asarray(inputs["w"], dtype=np.float32))
    assert x.shape == (N_CORES * IMGS, H, W, C), x.shape
    assert w.shape == (3, 3, C, O), w.shape

    nc = _get_nc()
    wq = prep_wq(w)
    ident = prep_ident()
    xr = x.reshape(N_CORES, IMGS, H * W, C)
    in_maps = [{"x": xr[i], "wq": wq, "ident": ident} for i in range(N_CORES)]
    if profile_dir is not None:
        hook = _ntff_hook()
        with hook(profile_dir, [0]):
            res = run_bass_kernel_spmd(nc, in_maps, list(range(N_CORES)))
    else:
        res = run_bass_kernel_spmd(nc, in_maps, list(range(N_CORES)))
    out = np.concatenate([res.results[i]["out"] for i in range(N_CORES)], axis=0)
    return out, res


def kernel(**inputs: np.ndarray) -> np.ndarray:
    out, _ = run(inputs)
    return out


# revision 42
# speedup vs baseline: 1.0494x; 1.0188x over previous
"""BinaryConv2D Trainium2 kernel — FP8 DoubleRow version.

Reference computation:
    out = conv2d(sign(x), sign(w), SAME, stride 1)   # sign(v) = +1 if v>=0 else -1
    x: (64, 56, 56, 128) f32, w: (3, 3, 128, 256) f32 -> out (64, 56, 56, 256) f32

Strategy (data-parallel over batch, 8 images per NeuronCore):
  1. Per image: SWDGE cast-DMA x f32 -> bf16 (HBM->HBM), then HW xbar
     DMA-transpose -> SBUF [128 ci, 3136 px] bf16.
  2. DVE binarize into TWO zero-padded fp8 planes per image (values +-0.5,
     weights are scaled +-2 so products are exactly +-1):
       A[r, c] = sign(x[r-1, c-1]) / 2    (the standard SAME-pad layout)
       B[r, c] = sign(x[r-1, c])   / 2    (shifted copy, for horizontal pairs)
     Plane rows are PW=64 wide so vertical tap offsets are 16-byte aligned
     (a DoubleRow AP requirement).
  3. Conv: weights STATIONARY in fp8 DoubleRow mode — each matmul contracts
     2 taps x 128 ci at once.  9 taps -> 4 DoubleRow pairs + 1 normal fp8
     matmul, accumulated in PSUM f32 [128 co_half, 512 px] per 8-row group:
       P0..P2: (0,dj)+(1,dj)  vertical pairs, moving-AP pair step 64 B
       P3:     (2,0)@A+(2,1)@B, pair step 3840 B (A->B plane offset)
       P4:     (2,2) normal fp8 matmul
     Moving operand = overlapping strided AP [128, 2, 512] over the plane.
  4. Output comes out transposed ([co, px]); Scalar engine copies PSUM ->
     bf16 (exact: |out| <= 512 in practice), then PE identity-matmul
     transposes flip each [co 128, px 112] chunk to [px, co]; DVE/ACT copy
     to an f32 stage and 4 DMAs per (image, co_half) write NHWC output.

Built on bacc.Bacc so multi-semaphore waits are legalized into
EventSemaphore chains.
"""

import sys

if "/opt/trn_rl_repo" not in sys.path:
    sys.path.insert(0, "/opt/trn_rl_repo")

import numpy as np
import bass_rust

import concourse.bacc as bacc
import concourse.bass as bass
import concourse.mybir as mybir
from concourse.tile import TileContext
from concourse.bass_utils import run_bass_kernel_spmd

N_CORES = 8
IMGS = 8  # images per core
H = W = 56
C = 128  # input channels (= contraction dim = SBUF partitions)
O = 256  # output channels
PW = 64  # padded row width (16-aligned for DoubleRow pair steps)
PROWS = 60  # 58 padded rows + 2 zero guard rows for tap reads past the end
PLANE = PROWS * PW  # 3840 bytes per partition per plane
GROUPS = 7  # 8-output-row groups per image
GW = 8 * PW  # 512 moving pixels per group
F32 = mybir.dt.float32
BF16 = mybir.dt.bfloat16
FP8 = mybir.dt.float8e4
DR = mybir.MatmulPerfMode.DoubleRow

# DoubleRow tap pairs (di, dj) and the single leftover tap
PAIRS = [((0, 0), (1, 0)), ((0, 1), (1, 1)), ((0, 2), (1, 2)), ((2, 0), (2, 1))]
SINGLE = (2, 2)


def _pair_rhs(plane_ap, off: int, step: int, n: int):
    """Overlapping 3D moving AP [128, 2, n]: two tap windows `step` bytes
    apart, each n contiguous fp8 pixels starting at `off` within the tile."""
    return bass_rust.AP(
        tensor=plane_ap.tensor,
        offset=plane_ap.offset + off,
        ap=[[plane_ap.ap[0][0], plane_ap.ap[0][1]], [step, 2], [1, n]],
    )


def build_nc() -> bass.Bass:
    nc = bacc.Bacc()
    x_t = nc.dram_tensor("x", [IMGS, H * W, C], F32, kind="ExternalInput")
    wq_t = nc.dram_tensor("wq", [C, 2304], FP8, kind="ExternalInput")
    id_t = nc.dram_tensor("ident", [128, 128], BF16, kind="ExternalInput")
    y_t = nc.dram_tensor("out", [IMGS, H, W, O], F32, kind="ExternalOutput")
    # per-image bf16 bounce tensors for the cast + xbar-transpose input path
    xb_ts = [nc.dram_tensor(f"xb{i}", [H * W, C], BF16) for i in range(IMGS)]

    with TileContext(nc) as tc:
        with (
            tc.tile_pool(name="const", bufs=1) as constp,
            tc.tile_pool(name="xtr", bufs=5) as xtrp,
            tc.tile_pool(name="out1", bufs=6) as out1p,
            tc.tile_pool(name="stage", bufs=4) as stagep,
            tc.tile_pool(name="psum1", bufs=6, space="PSUM") as psum1p,
            tc.tile_pool(name="psum2", bufs=2, space="PSUM") as psum2p,
        ):
            wq = constp.tile([C, 2304], FP8)
            nc.sync.dma_start(out=wq[:], in_=wq_t[:])
            identb = constp.tile([128, 128], BF16)
            nc.sync.dma_start(out=identb[:], in_=id_t[:])

            planes = []
            for i in range(IMGS):
                plane = constp.tile([C, 2 * PLANE], FP8, tag=f"plane{i}")
                planes.append(plane)

            xtrs = {}
            HALF = H * W // 2  # 1568 px = 28 image rows

            def prep_input(i, halves=False):
                """cast-DMA + xbar transpose image i (non-blocking queues).
                halves=True splits into two half-image pipelines so the first
                matmuls can start sooner (image 0 startup latency)."""
                if halves:
                    nc.gpsimd.dma_start(out=xb_ts[i][0:HALF], in_=x_t[i][0:HALF])
                    nc.gpsimd.dma_start(
                        out=xb_ts[i][HALF : H * W], in_=x_t[i][HALF : H * W]
                    )
                    xtra = xtrp.tile([C, HALF], BF16)
                    nc.sync.dma_start(
                        out=xtra[:], in_=xb_ts[i][0:HALF], transpose=True
                    )
                    xtrb = xtrp.tile([C, HALF], BF16)
                    nc.sync.dma_start(
                        out=xtrb[:], in_=xb_ts[i][HALF : H * W], transpose=True
                    )
                    xtrs[i] = (xtra, xtrb)
                    return
                nc.gpsimd.dma_start(out=xb_ts[i][:], in_=x_t[i])
                xtr = xtrp.tile([C, H * W], BF16)
                nc.sync.dma_start(out=xtr[:], in_=xb_ts[i][:], transpose=True)
                xtrs[i] = xtr

            def prep_plane(i):
                """Pad-zero + binarize image i, split across DVE (plane A)
                and ACT (plane B) so the two run concurrently.  Only pad
                regions that valid outputs actually read are zeroed; junk
                columns 58+ and the guard rows only ever feed discarded
                output columns, so stale SBUF there is harmless."""
                xtr = xtrs.pop(i)
                pl = planes[i]
                intA = pl[:, PW : PW + 56 * PW].rearrange("c (r w) -> c r w", w=PW)
                intB = pl[:, PLANE + PW : PLANE + PW + 56 * PW].rearrange(
                    "c (r w) -> c r w", w=PW
                )
                # A pads on DVE: top row, bottom rows + guards, side cols
                nc.vector.memset(pl[:, 0:PW], 0.0)
                nc.vector.memset(pl[:, 57 * PW : PLANE], 0.0)
                nc.vector.memset(intA[:, :, 0:1], 0.0)
                nc.vector.memset(intA[:, :, 57:64], 0.0)
                # B pads on ACT (MemsetZero still reads its input AP, so
                # feed it a broadcast of the always-initialized identity)
                def act_zero(dst):
                    ib = identb[:]
                    bc = bass_rust.AP(
                        tensor=ib.tensor,
                        offset=ib.offset,
                        ap=[[ib.ap[0][0], dst.ap[0][1]]]
                        + [[0, n] for _, n in dst.ap[1:]],
                    )
                    nc.scalar.activation(
                        dst, bc, mybir.ActivationFunctionType.Copy, scale=0.0
                    )

                act_zero(pl[:, PLANE + 57 * PW : 2 * PLANE])
                act_zero(intB[:, :, 56:64])
                act_zero(pl[:, PLANE : PLANE + PW])

                pieces = (
                    [(xtr, 0, H)]
                    if not isinstance(xtr, tuple)
                    else [(xtr[0], 0, H // 2), (xtr[1], H // 2, H)]
                )
                for piece, r0, r1 in pieces:
                    src = piece[:].rearrange("c (r w) -> c r w", w=W)
                    nc.vector.tensor_scalar(
                        intA[:, r0:r1, 1 : 1 + W],
                        src,
                        0.0,
                        0.5,
                        op0=mybir.AluOpType.is_ge,
                        op1=mybir.AluOpType.subtract,
                    )
                    # plane B binarize on the Scalar engine: Sign gives +-1
                    # (not +-0.5), so the B-side tap weights are +-1
                    nc.scalar.activation(
                        intB[:, r0:r1, 0:W],
                        src,
                        mybir.ActivationFunctionType.Sign,
                    )

            prep_input(0, halves=True)
            prep_input(1)
            prep_plane(0)

            for i in range(IMGS):
                if i + 2 < IMGS:
                    prep_input(i + 2)
                pl_ap = planes[i][:]
                for h in range(2):
                    if h == 0 and i + 1 < IMGS:
                        prep_plane(i + 1)
                    stage = stagep.tile([128, 28 * 128], F32)
                    st3 = stage[:].rearrange("p (t o) -> p t o", o=128)
                    pending = None  # (g, out1 tile) awaiting transpose

                    def flush_transposes():
                        nonlocal pending
                        if pending is None:
                            return
                        g, o1 = pending
                        pending = None
                        ps2 = psum2p.tile([128, GW], BF16)
                        for c in range(4):
                            nc.tensor.transpose(
                                ps2[:, c * 128 : (c + 1) * 128],
                                o1[:, c * 128 : (c + 1) * 128],
                                identb[:],
                            )
                        # one whole-group drain (DVE); psum1 drains are ACT
                        nc.vector.tensor_copy(
                            stage[:, g * GW : (g + 1) * GW], ps2[:]
                        )

                    def out_dmas(c0, c1):
                        # (even/odd rows) x chunk range; on gpsimd to keep
                        # the sync queue free for the input xbar transposes
                        ye = y_t[i].rearrange("(r2 two) w c -> two w r2 c", two=2)
                        cs = slice(c0, c1)
                        nc.gpsimd.dma_start(
                            out=ye[0][:, cs, h * 128 : (h + 1) * 128],
                            in_=st3[0:W, cs, :],
                        )
                        nc.gpsimd.dma_start(
                            out=ye[1][:, cs, h * 128 : (h + 1) * 128],
                            in_=st3[64 : 64 + W, cs, :],
                        )

                    last = i == IMGS - 1 and h == 1
                    for g in range(GROUPS):
                        ps1 = psum1p.tile([128, GW], F32)
                        base = PW * 8 * g
                        for p in range(len(PAIRS)):
                            lhsT = wq[
                                :, (p * 2 + h) * 256 : (p * 2 + h) * 256 + 256
                            ].rearrange("c (t m) -> c t m", t=2)
                            if p < 3:
                                rhs = _pair_rhs(pl_ap, base + p, PW, GW)
                            else:
                                rhs = _pair_rhs(pl_ap, base + 2 * PW, PLANE, GW)
                            nc.tensor.matmul(
                                ps1[:], lhsT, rhs, start=(p == 0), stop=False,
                                perf_mode=DR,
                            )
                        nc.tensor.matmul(
                            ps1[:],
                            wq[:, 2048 + h * 128 : 2048 + (h + 1) * 128],
                            pl_ap[:, base + 2 * PW + 2 : base + 2 * PW + 2 + GW],
                            start=False,
                            stop=True,
                        )
                        o1 = out1p.tile([128, GW], BF16)
                        nc.scalar.activation(
                            o1[:], ps1[:], mybir.ActivationFunctionType.Copy
                        )
                        flush_transposes()
                        pending = (g, o1)
                        # drain staged chunks early; finer-grained on the
                        # very last half-image to shorten the kernel tail
                        if last and g in (2, 4, 6):
                            out_dmas(7 * (g // 2 - 1), 7 * (g // 2))
                        elif not last and g == 4:
                            out_dmas(0, 14)
                    flush_transposes()
                    if last:
                        out_dmas(21, 28)
                    else:
                        out_dmas(14, 28)

    nc.finalize()
    return nc


_NC_CACHE = None


def _get_nc():
    global _NC_CACHE
    if _NC_CACHE is None:
        _NC_CACHE = build_nc()
    return _NC_CACHE


def prep_wq(w: np.ndarray) -> np.ndarray:
    """Binarize weights to +-2 fp8 (inputs are +-0.5 -> products +-1),
    laid out [ci, pair/co_half/tap/co_low] for DoubleRow stationary loads."""
    import ml_dtypes

    wb = np.where(w >= 0, np.float32(2.0), np.float32(-2.0))  # [3,3,128,256]
    wb1 = np.where(w >= 0, np.float32(1.0), np.float32(-1.0))
    cols = np.zeros((C, 2304), np.float32)
    for p, (t0, t1) in enumerate(PAIRS):
        # pair 3's second tap reads plane B, whose values are +-1 (Sign
        # activation), not +-0.5 -> its weights stay +-1
        w1 = wb1 if p == 3 else wb
        for h in range(2):
            base = (p * 2 + h) * 256
            cols[:, base : base + 128] = wb[t0[0], t0[1], :, h * 128 : (h + 1) * 128]
            cols[:, base + 128 : base + 256] = w1[
                t1[0], t1[1], :, h * 128 : (h + 1) * 128
            ]
    for h in range(2):
        cols[:, 2048 + h * 128 : 2048 + (h + 1) * 128] = wb[
            SINGLE[0], SINGLE[1], :, h * 128 : (h + 1) * 128
        ]
    return np.ascontiguousarray(cols.astype(ml_dtypes.float8_e4m3))


def prep_ident() -> np.ndarray:
    import ml_dtypes

    return np.eye(128, dtype=np.float32).astype(ml_dtypes.bfloat16)


def _ntff_hook():
    """NTFF capture context manager via the axon PJRT .so."""
    sys.path.insert(0, "/root/.axon_site")
    from trn_agent_boot.trn_boot import _ntff_profile_via_ctypes

    return _ntff_profile_via_ctypes("/opt/axon/libaxon_pjrt.so")


def run(inputs: dict, profile_dir: str | None = None):
    """Run on all 8 NeuronCores. Returns (full_output, BassKernelResults)."""
    x = np.ascontiguousarray(np.asarray(inputs["x"], dtype=np.float32))
    w = np.ascontiguousarray(np.asarray(inputs["w"], dtype=np.float32))
    assert x.shape == (N_CORES * IMGS, H, W, C), x.shape
    assert w.shape == (3, 3, C, O), w.shape

    nc = _get_nc()
    wq = prep_wq(w)
    ident = prep_ident()
    xr = x.reshape(N_CORES, IMGS, H * W, C)
    in_maps = [{"x": xr[i], "wq": wq, "ident": ident} for i in range(N_CORES)]
    if profile_dir is not None:
        hook = _ntff_hook()
        with hook(profile_dir, [0]):
            res = run_bass_kernel_spmd(nc, in_maps, list(range(N_CORES)))
    else:
        res = run_bass_kernel_spmd(nc, in_maps, list(range(N_CORES)))
    out = np.concatenate([res.results[i]["out"] for i in range(N_CORES)], axis=0)
    return out, res


def kernel(**inputs: np.ndarray) -> np.ndarray:
    out, _ = run(inputs)
    return out
